# revision 35
# baseline (speedup 1.0000x reference)
"""AttentionWithPairBias distributed Trainium2 kernel (8 NeuronCores).

Sequence-parallel: core c owns query rows i in [128c, 128(c+1)).
Per core: z shard [128, 1024, 128] f32 (64MB -> the memory roofline),
s and weights replicated. No collectives.

v2 (trace-driven rewrite of the 453-556us baseline):
- The baseline serialized the f32->bf16 z cast DMA (HBM-bound,
  ~22us/jt) with the SBUF->SBUF xbar transpose (~19us/jt) because both
  use all 16 DMA engines and the xbar mode switch drains in-flight
  DMAs -> ~45us/jt. Fix: transpose z on the PE instead (128
  is_transpose matmuls per jt, ~7us at 2.4GHz) with PSUM->SBUF drains
  split across DVE/ScalarE/Pool. The DMA engines then carry ONLY the
  casts, back-to-back at the HBM roofline (~22.3us/jt).
- All aux loads (s, weights, mask) ride the sync-ring HWDGE, which no
  longer carries z transposes, so they overlap the SWDGE casts.
- z^2 stats: direct tensor_reduce per 32-j chunk (DVE 2x modes) with
  squares split ScalarE/DVE; no halving tree.
- Bias scale (rsqrt mult) + additive mask ride the idle gpsimd/Pool
  engine.
- v/g projections run in z-phase PE slack instead of the tail.
- Tail unchanged in structure: per 4-head batch qk (PE) -> fused bias
  add (DVE) -> exp with accum denominator (ScalarE) -> normalize ->
  batched xbar transpose (DMA idle in tail) -> attn@v -> out proj.
Rel err ~7e-3 (gate 2e-2), same numeric profile as baseline (bf16
s/weights/B).
"""

import os
from contextlib import ExitStack

import numpy as np

import concourse.bass as bass
import concourse.bacc as bacc
import concourse.tile as tile
import concourse.mybir as mybir
from concourse.masks import make_identity

S = 1024
CS = 384
CZ = 128
D = 32
H = 12
NCORES = 8
RB = S // NCORES  # 128 query rows per core
JT = S // 128     # 8 column tiles
CKS = CS // 128   # 3 contraction chunks of s-dim
EPS = 1e-5
INVD = 1.0 / np.sqrt(D)

F32 = mybir.dt.float32
BF16 = mybir.dt.bfloat16
F16 = mybir.dt.float16
I32 = mybir.dt.int32
AF = mybir.ActivationFunctionType
OP = mybir.AluOpType

BI = 32   # bias psum batch (32*12*4B = 1.5KB <= bank)
SB = 16   # stats chunk (j per square/reduce pass)


def _mm(nc, out, lhsT, rhs, start, stop, **kw):
    nc.tensor.matmul(out, lhsT, rhs, start=start, stop=stop, **kw)


def build(nc):
    s_full = nc.dram_tensor("s", [S, CS], F32, kind="ExternalInput").ap()
    s_loc = nc.dram_tensor("s_loc", [RB, CS], F32, kind="ExternalInput").ap()
    z_d = nc.dram_tensor("z", [RB, S, CZ], F32, kind="ExternalInput").ap()
    zm_d = nc.dram_tensor("z_mask", [RB, S], I32, kind="ExternalInput").ap()
    ws_d = nc.dram_tensor("w_s", [CS], F32, kind="ExternalInput").ap()
    wz_d = nc.dram_tensor("w_z", [CZ], F32, kind="ExternalInput").ap()
    Wz_d = nc.dram_tensor("Wz", [CZ, H], F32, kind="ExternalInput").ap()
    Wq_d = nc.dram_tensor("Wq", [CS, CS], F32, kind="ExternalInput").ap()
    Wk_d = nc.dram_tensor("Wk", [CS, CS], F32, kind="ExternalInput").ap()
    Wv_d = nc.dram_tensor("Wv", [CS, CS], F32, kind="ExternalInput").ap()
    Wg_d = nc.dram_tensor("Wg", [CS, CS], F32, kind="ExternalInput").ap()
    bg_d = nc.dram_tensor("bg", [CS], F32, kind="ExternalInput").ap()
    Wo_d = nc.dram_tensor("Wo", [CS, CS], F32, kind="ExternalInput").ap()
    bo_d = nc.dram_tensor("bo", [CS], F32, kind="ExternalInput").ap()
    out_d = nc.dram_tensor("out", [RB, CS], F32, kind="ExternalOutput").ap()

    with tile.TileContext(nc) as tc, ExitStack() as ctx:
        sg = ctx.enter_context(tc.tile_pool(name="singles", bufs=1))
        # preamble + tail-persistent pools outlive the z pools (LIFO)
        pctx = ExitStack()
        pre = pctx.enter_context(tc.tile_pool(name="pre", bufs=1))
        pp = pctx.enter_context(tc.tile_pool(name="pre_ps", bufs=2,
                                             space="PSUM"))
        tlw = pctx.enter_context(tc.tile_pool(name="tailw", bufs=1))
        zctx = ExitStack()
        znp = zctx.enter_context(tc.tile_pool(name="znat", bufs=4))
        ztp = zctx.enter_context(tc.tile_pool(name="znT", bufs=1))
        sqp = zctx.enter_context(tc.tile_pool(name="sqp", bufs=2))
        trp = zctx.enter_context(tc.tile_pool(name="trp", bufs=2))
        msp = zctx.enter_context(tc.tile_pool(name="msp", bufs=2))
        wpl = zctx.enter_context(tc.tile_pool(name="wpool", bufs=1))
        tpp = zctx.enter_context(tc.tile_pool(name="tr_ps", bufs=2,
                                              space="PSUM"))
        bpp = zctx.enter_context(tc.tile_pool(name="bias_ps", bufs=2,
                                              space="PSUM"))

        zn_tiles = {}

        def issue_cast(jt, half=None):
            # half-tile casts: znI slots recycle at 64-j granularity so a
            # late stats chain on one half doesn't stall the cast FIFO
            if half is None:
                issue_cast(jt, 0)
                issue_cast(jt, 1)
                return
            znI = znp.tile([128, 64, CZ], BF16, tag="zn", name="znI")
            nc.gpsimd.dma_start(
                out=znI, in_=z_d[:, jt * 128 + half * 64:
                                 jt * 128 + (half + 1) * 64, :])
            zn_tiles[(jt, half)] = znI

        # all aux + casts ride the SWDGE FIFO in explicit order: the aux
        # bytes are on the shared-HBM critical path no matter which ring
        # carries them, and HWDGE transfers starve behind 16-engine SWDGE
        # bursts, so ordering one FIFO is the robust choice. s first
        # (pre_s1 fills the cast(0) window), weights spread between casts.
        s8_f = sg.tile([128, JT, CS], F32)
        nc.gpsimd.dma_start(
            out=s8_f, in_=s_full.rearrange("(t p) c -> p t c", p=128))
        sl_f = sg.tile([128, CS], F32)
        nc.gpsimd.dma_start(out=sl_f, in_=s_loc)
        issue_cast(0)

        # ---------- consts + tiny aux (SWDGE, before cast(1)) ----------
        ident_b = sg.tile([128, 128], BF16)
        make_identity(nc, ident_b)
        ones1 = sg.tile([1, 128], F32)
        nc.vector.memset(ones1, 1.0)
        eps_t = sg.tile([128, 1], F32)
        nc.vector.memset(eps_t, EPS)

        Wz_sb = sg.tile([128, H], F32)
        nc.gpsimd.dma_start(out=Wz_sb, in_=Wz_d)
        wzv_sb = sg.tile([128, 1], F32)
        nc.gpsimd.dma_start(out=wzv_sb, in_=wz_d.rearrange("(p o) -> p o", o=1))
        ws_sb = sg.tile([128, CKS], F32)
        nc.gpsimd.dma_start(out=ws_sb, in_=ws_d.rearrange("(k p) -> p k", p=128))
        bg_sb = sg.tile([1, CS], F32)
        nc.gpsimd.dma_start(out=bg_sb, in_=bg_d.rearrange("(o c) -> o c", o=1))
        bo_sb = sg.tile([1, CS], F32)
        nc.gpsimd.dma_start(out=bo_sb, in_=bo_d.rearrange("(o c) -> o c", o=1))

        nc.vector.tensor_scalar_mul(Wz_sb, Wz_sb, wzv_sb)
        Wz_bf = sg.tile([128, H], BF16)
        nc.vector.tensor_copy(out=Wz_bf, in_=Wz_sb)

        # persisted bias tiles [i, jt, j, h] + additive attention mask
        B_all = sg.tile([128, JT, 128, H], BF16)
        maskneg = sg.tile([128, S], BF16)
        mi_sb = sg.tile([128, S], I32)

        def mask_add_park(jt, eng):
            B_loc = B_all[:, jt, :, :]
            mslice = bass.AP(
                tensor=maskneg.tensor, offset=maskneg.offset + jt * 128,
                ap=[maskneg.ap[0], [1, 128], [0, H]])
            eng.tensor_tensor(out=B_loc, in0=B_loc, in1=mslice, op=OP.add)

        # weight loads: SWDGE cast-DMAs (f32 -> bf16 in the DMA), placed
        # in the FIFO between z casts by call order
        def load_w(dram, pool, name="wb"):
            wb = pool.tile([128, CKS, CS], BF16, tag=name, name=name)
            nc.gpsimd.dma_start(
                out=wb, in_=dram.rearrange("(k p) c -> p k c", p=128))
            return wb

        # ---------- per-jt z pipeline ----------
        # Measured engine rates drive this split: ScalarE Square is the
        # only fast squaring path (DVE self-mult pays a 2-read penalty),
        # DVE tree adds on distinct slices get ~2x, f32-out reduces are
        # stuck at 1x, PSUM drains are cheapest on DVE (2x), and Pool
        # cannot touch PSUM at all (its sole heavy job: one square chunk).
        def z_step(jt):
            B_loc = B_all[:, jt, :, :]
            zh = [zn_tiles.pop((jt, 0)), zn_tiles.pop((jt, 1))]
            # stats first in program order: squares have no PE dependency,
            # so they can't get stuck behind the engines' drain waits.
            # 8 chunks of 16 j, sq slots 4 deep to decouple the chain.
            msI = msp.tile([128, 128], F32, tag="msI", name="msI")
            for q in range(8):
                sq = sqp.tile([128, SB, CZ], BF16, tag="sq", name="sq")
                src = zh[q // 4][:, bass.ts(q % 4, SB), :]
                if q >= 6:
                    nc.gpsimd.tensor_tensor(out=sq, in0=src, in1=src,
                                            op=OP.mult)
                else:
                    nc.scalar.square(out=sq, in_=src)
                # tree stages write SEPARATE tiles: 3-AP-same-tile in-place
                # ops fall off the DVE fast path (~4x slower, measured)
                tA = trp.tile([128, SB, 64], BF16, tag="tA", name="tA")
                tA_eng = nc.gpsimd if q % 2 == 0 else nc.vector
                tA_eng.tensor_tensor(
                    out=tA, in0=sq[:, :, 0:64], in1=sq[:, :, 64:128],
                    op=OP.add)
                tB = trp.tile([128, SB, 32], BF16, tag="tB", name="tB")
                nc.vector.tensor_tensor(
                    out=tB, in0=tA[:, :, 0:32], in1=tA[:, :, 32:64],
                    op=OP.add)
                nc.vector.tensor_reduce(
                    out=msI[:, bass.ts(q, SB)], in_=tB,
                    axis=mybir.AxisListType.X, op=OP.add)
                if q % 2 == 1:
                    # rsqrt per 32-j pair so bias scaling never waits the
                    # whole stats pass
                    mpair = msI[:, (q - 1) * SB:(q + 1) * SB]
                    nc.scalar.activation(out=mpair, in_=mpair, func=AF.Sqrt,
                                         bias=eps_t, scale=float(1.0 / CZ))
                    nc.vector.reciprocal(out=mpair, in_=mpair)
            # PE transposes per j-slice -> PSUM (bf16), drain to zt
            zt = ztp.tile([128, 128, 128], BF16, tag="zt", name="zt")
            for bg in range(8):
                ps = tpp.tile([128, 16, 128], BF16, tag="tps", name="tps")
                for jj in range(16):
                    _mm(nc, ps[:, jj, :], zh[bg // 4][:, (bg % 4) * 16 + jj, :],
                        ident_b, True, True, is_transpose=True)
                dst = zt[:, bg * 16:(bg + 1) * 16, :]
                if bg % 2 == 1:
                    nc.scalar.copy(out=dst, in_=ps)
                else:
                    nc.vector.tensor_copy(out=dst, in_=ps)
            # cast(jt+2): the znI-slot waits ride the Pool engine; h1 comes
            # after Pool's own h1 squares in Pool program order (q6/q7)
            if jt + 2 < JT:
                issue_cast(jt + 2, 0)
                issue_cast(jt + 2, 1)
            # bias matmuls: B[i, j, h] = zt[:, j, :].T @ Wz
            for b in range(RB // BI):
                j0 = b * BI
                b_ps = bpp.tile([128, BI, H], F32, tag="bps", name="b_ps")
                for jj in range(BI):
                    _mm(nc, b_ps[:, jj, :], zt[:, j0 + jj, :], Wz_bf,
                        jj == 0, jj == BI - 1)
                rs_b = bass.AP(
                    tensor=msI.tensor,
                    offset=msI.offset + j0,
                    ap=[msI.ap[0], [1, BI], [0, H]])
                nc.vector.tensor_tensor(
                    out=B_loc[:, j0:j0 + BI, :], in0=b_ps, in1=rs_b,
                    op=OP.mult)
            return B_loc

        # ---------- preamble (fills the cast(0)/cast(1) window) ----------
        s_rT = pre.tile([128, CKS, S], BF16)     # [c, k, i]
        s_rTl = pre.tile([128, CKS, 128], BF16)  # [c, k, local i]
        kT = pre.tile([128, CKS, S], BF16)       # [hd_in_chunk, chunk, j]
        qT = pre.tile([128, CKS, 128], BF16)     # [hd_in_chunk, chunk, i_loc]
        _ncopy = [0]

        def drain(dst, psrc):
            if _ncopy[0] % 2 == 0:
                nc.vector.tensor_copy(out=dst, in_=psrc)
            else:
                nc.scalar.copy(out=dst, in_=psrc)
            _ncopy[0] += 1

        def pre_s1():
            with tc.tile_pool(name="pre_tmp", bufs=2) as pt:
                def norm_rows(ap, out_bf):
                    nsq = pt.tile([128, CS], BF16, tag="nsq")
                    msum = pt.tile([128, 1], F32, tag="nms")
                    nc.scalar.activation(out=nsq, in_=ap, func=AF.Square,
                                         scale=float(1.0 / np.sqrt(CS)),
                                         accum_out=msum)
                    nc.scalar.activation(out=msum, in_=msum, func=AF.Sqrt,
                                         bias=eps_t, scale=1.0)
                    nc.vector.reciprocal(out=msum, in_=msum)
                    nc.vector.scalar_tensor_tensor(
                        out=out_bf, in0=ap, scalar=1.0,
                        in1=bass.AP(tensor=msum.tensor, offset=msum.offset,
                                    ap=[msum.ap[0], [0, CS]]),
                        op0=OP.mult, op1=OP.mult)

                def tr128(dst, src):
                    # psum slot shared with tag "big" (same 2KB bank)
                    ps = pp.tile([128, 1024], BF16, tag="big")
                    _mm(nc, ps[:, 0:128], src, ident_b,
                        True, True, is_transpose=True)
                    drain(dst, ps[:, 0:128])

                for t in range(JT):
                    s_rb = pt.tile([128, CS], BF16, tag="s_rb")
                    norm_rows(s8_f[:, t, :], s_rb)
                    for k in range(CKS):
                        tr128(s_rT[:, k, bass.ts(t, 128)],
                              s_rb[:, bass.ts(k, 128)])
                s_rlb = pt.tile([128, CS], BF16, tag="s_rb")
                norm_rows(sl_f, s_rlb)
                for k in range(CKS):
                    tr128(s_rTl[:, k, :], s_rlb[:, bass.ts(k, 128)])
            for k in range(CKS):
                nc.vector.tensor_scalar_mul(
                    s_rT[:, k, :], s_rT[:, k, :], ws_sb[:, k:k + 1])
                nc.vector.tensor_scalar_mul(
                    s_rTl[:, k, :], s_rTl[:, k, :], ws_sb[:, k:k + 1])

        wbox = {}

        def pre_qt():
            wb = wbox.pop("wq")
            for k in range(CKS):
                ps = pp.tile([128, 512], F32, tag="big")
                for ck in range(CKS):
                    _mm(nc, ps[:, 0:128], wb[:, ck, bass.ts(k, 128)],
                        s_rTl[:, ck, :], ck == 0, ck == CKS - 1)
                nc.scalar.mul(out=qT[:, k, :], in_=ps[:, 0:128],
                              mul=float(INVD))

        def pre_kt(ks):
            wb = wbox["wk"]
            for k in ks:
                for half in range(2):
                    ps2 = pp.tile([128, 512], F32, tag="big")
                    for ck in range(CKS):
                        _mm(nc, ps2, wb[:, ck, bass.ts(k, 128)],
                            s_rT[:, ck, bass.ts(half, 512)], ck == 0,
                            ck == CKS - 1)
                    drain(kT[:, k, bass.ts(half, 512)], ps2)

        # tail-persistent tensors
        v_sb = tlw.tile([128, JT, CS], BF16)  # [j_in_tile, jt, hd]
        g_sb = tlw.tile([128, CS], F32)
        wo_f = tlw.tile([128, CKS, CS], F32, name="w_Wo")
        oT_sb = tlw.tile([128, CKS, 128], F32)  # [hd_in_chunk, chunk, i]

        def v_and_g():
            wb = wbox["wv"]
            for jc in range(JT):
                ps2 = pp.tile([128, 512], F32, tag="big")
                for ck in range(CKS):
                    _mm(nc, ps2[:, 0:CS], s_rT[:, ck, bass.ts(jc, 128)],
                        wb[:, ck, :], ck == 0, ck == CKS - 1)
                drain(v_sb[:, jc, :], ps2[:, 0:CS])
            wbg = wbox["wg"]
            ps2 = pp.tile([128, 512], F32, tag="big")
            for ck in range(CKS):
                _mm(nc, ps2[:, 0:CS], s_rTl[:, ck, :], wbg[:, ck, :],
                    ck == 0, False)
            _mm(nc, ps2[:, 0:CS], ones1, bg_sb, False, True)
            nc.scalar.copy(out=g_sb, in_=ps2[:, 0:CS])

        # SWDGE FIFO continues: wq, wk, cast(1), wv, wg, wo, mask, casts
        wbox["wq"] = load_w(Wq_d, wpl, name="wq")
        wbox["wk"] = load_w(Wk_d, wpl, name="wk")
        issue_cast(1)
        wbox["wv"] = load_w(Wv_d, wpl, name="wv")
        wbox["wg"] = load_w(Wg_d, wpl, name="wg")
        nc.gpsimd.dma_start(
            out=wo_f, in_=Wo_d.rearrange("(k p) c -> p k c", p=128))
        nc.gpsimd.dma_start(out=mi_sb, in_=zm_d)
        nc.vector.tensor_scalar(
            out=maskneg, in0=mi_sb, scalar1=0, scalar2=-30000.0,
            op0=OP.is_equal, op1=OP.mult)

        # preamble compute while cast(0)/cast(1) stream in
        pre_s1()
        pre_qt()

        # ---------- z steps; kT built in early-step slack, mask parks
        # (Pool) once the mask has landed ----------
        stages = {0: lambda: pre_kt([0]), 1: lambda: pre_kt([1]),
                  2: lambda: pre_kt([2]),
                  4: lambda: [mask_add_park(t, nc.gpsimd) for t in (0, 1, 2)],
                  5: v_and_g,
                  6: lambda: [mask_add_park(t, nc.gpsimd) for t in (3, 4, 5, 6)]}
        for jt in range(JT):
            z_step(jt)
            if jt in stages:
                stages[jt]()
        mask_add_park(7, nc.gpsimd)
        zctx.close()

        # ---------- scores + attention tail ----------
        # bias-add split by head parity: even heads accumulate B into the
        # qk PSUM on the PE (exp reads PSUM), odd heads do a DVE STT into
        # an f16 scores tile (exp reads that) — balances PE vs DVE, both
        # ~50% loaded in the tail.
        HB = 4   # heads per transpose batch
        with tc.tile_pool(name="att_sb", bufs=2) as asb, \
             tc.tile_pool(name="attT_sb", bufs=2) as atsb, \
             tc.tile_pool(name="att16", bufs=2) as a16p, \
             tc.tile_pool(name="den_sb", bufs=2) as dsb, \
             tc.tile_pool(name="qk_ps", bufs=2, space="PSUM") as qkp, \
             tc.tile_pool(name="o_ps", bufs=2, space="PSUM") as opp:
            for batch in range(H // HB):
                attb = asb.tile([128, HB, JT, 128], BF16, tag="attb",
                                name="attb")
                for hh in range(HB):
                    h = batch * HB + hh
                    ck, hp = divmod(h, 4)
                    qk = qkp.tile([128, S], F32, tag="qk", name="qk")
                    pe_bias = (h % 2 == 0)
                    for half in range(2):
                        _mm(nc, qk[:, bass.ts(half, 512)],
                            qT[bass.ts(hp, 32), ck, :],
                            kT[bass.ts(hp, 32), ck, bass.ts(half, 512)],
                            True, not pe_bias, tile_position=(32 * hp, 0))
                        if pe_bias:
                            b_half = bass.AP(
                                tensor=B_all.tensor,
                                offset=B_all.offset + h + half * 4 * (128 * H),
                                ap=[B_all.ap[0], [128 * H, 4], [H, 128]])
                            _mm(nc, qk[:, bass.ts(half, 512)], ident_b,
                                b_half, False, True)
                    att = attb[:, hh, :, :]
                    den = dsb.tile([128, 1], F32, tag="den", name="den")
                    if pe_bias:
                        exp_src = qk.rearrange("p (t j) -> p t j", j=128)
                    else:
                        att16 = a16p.tile([128, S], F16, tag="a16",
                                          name="att16")
                        b_slice = bass.AP(
                            tensor=B_all.tensor,
                            offset=B_all.offset + h,
                            ap=[B_all.ap[0], [128 * H, JT], [H, 128]])
                        qk3 = bass.AP(
                            tensor=qk.tensor, offset=qk.offset,
                            ap=[qk.ap[0], [128, JT], [1, 128]])
                        a163 = bass.AP(
                            tensor=att16.tensor, offset=att16.offset,
                            ap=[att16.ap[0], [128, JT], [1, 128]])
                        nc.vector.scalar_tensor_tensor(
                            out=a163, in0=qk3, scalar=1.0,
                            in1=b_slice, op0=OP.mult, op1=OP.add)
                        exp_src = att16.rearrange("p (t j) -> p t j", j=128)
                    nc.scalar.activation(out=att, in_=exp_src, func=AF.Exp,
                                         accum_out=den)
                    nc.vector.reciprocal(out=den, in_=den)
                    nc.vector.tensor_scalar_mul(att, att, den)
                atT = atsb.tile([128, HB, JT, 128], BF16, tag="atT",
                                name="atT")
                nc.sync.dma_start(out=atT, in_=attb, transpose=True)
                for hh in range(HB):
                    h = batch * HB + hh
                    o_ps = opp.tile([32, 128], F32, tag="o", name="o_ps")
                    for jc in range(JT):
                        _mm(nc, o_ps, v_sb[:, jc, bass.ts(h, 32)],
                            atT[:, hh, jc, :], jc == 0, jc == JT - 1)
                    ck, hp = divmod(h, 4)
                    if h % 2 == 0:
                        nc.vector.tensor_copy(
                            out=oT_sb[bass.ts(hp, 32), ck, :], in_=o_ps)
                    else:
                        nc.scalar.copy(out=oT_sb[bass.ts(hp, 32), ck, :],
                                       in_=o_ps)

            # ---------- output ----------
            fin = pp.tile([128, 512], F32, tag="big")
            for k in range(CKS):
                _mm(nc, fin[:, 0:CS], oT_sb[:, k, :], wo_f[:, k, :],
                    k == 0, False)
            _mm(nc, fin[:, 0:CS], ones1, bo_sb, False, True)
            out_sb = tlw.tile([128, CS], F32)
            nc.vector.tensor_tensor(out=out_sb, in0=fin[:, 0:CS], in1=g_sb,
                                    op=OP.mult)
            nc.sync.dma_start(out=out_d, in_=out_sb)
        pctx.close()

    nc.compile()
    return nc


_NC_CACHE = None


def _get_nc():
    global _NC_CACHE
    if _NC_CACHE is None:
        nc = bacc.Bacc("TRN2", target_bir_lowering=False, debug=False,
                       enable_asserts=False)
        _NC_CACHE = build(nc)
    return _NC_CACHE


def make_in_maps(s, z, z_mask, w_s, w_z, Wz, Wq, Wk, Wv, Wg, bg, Wo, bo):
    f = lambda a: np.ascontiguousarray(np.asarray(a), dtype=np.float32)
    s = f(s)
    shared = dict(s=s, w_s=f(w_s), w_z=f(w_z), Wz=f(Wz), Wq=f(Wq), Wk=f(Wk),
                  Wv=f(Wv), Wg=f(Wg), bg=f(bg), Wo=f(Wo), bo=f(bo))
    zmask = np.ascontiguousarray(np.asarray(z_mask), dtype=np.int32)
    z = f(z)
    in_maps = []
    for c in range(NCORES):
        r0, r1 = c * RB, (c + 1) * RB
        m = dict(shared)
        m["s_loc"] = np.ascontiguousarray(s[r0:r1])
        m["z"] = np.ascontiguousarray(z[r0:r1])
        m["z_mask"] = np.ascontiguousarray(zmask[r0:r1])
        in_maps.append(m)
    return in_maps


def kernel(**inputs):
    from concourse import bass_utils
    nc = _get_nc()
    in_maps = make_in_maps(**inputs)
    res = bass_utils.run_bass_kernel_spmd(nc, in_maps, core_ids=list(range(NCORES)))
    out = np.concatenate([res.results[c]["out"] for c in range(NCORES)], axis=0)
    return out.astype(np.float32)


# revision 36
# speedup vs baseline: 1.1511x; 1.1511x over previous
"""AttentionWithPairBias distributed Trainium2 kernel (8 NeuronCores).

Sequence-parallel: core c owns query rows i in [128c, 128(c+1)).
Per core: z shard [128, 1024, 128] f32 (64MB -> the memory roofline),
s and weights replicated. No collectives.

v2 (trace-driven rewrite of the 453-556us baseline):
- The baseline serialized the f32->bf16 z cast DMA (HBM-bound,
  ~22us/jt) with the SBUF->SBUF xbar transpose (~19us/jt) because both
  use all 16 DMA engines and the xbar mode switch drains in-flight
  DMAs -> ~45us/jt. Fix: transpose z on the PE instead (128
  is_transpose matmuls per jt, ~7us at 2.4GHz) with PSUM->SBUF drains
  split across DVE/ScalarE/Pool. The DMA engines then carry ONLY the
  casts, back-to-back at the HBM roofline (~22.3us/jt).
- All aux loads (s, weights, mask) ride the sync-ring HWDGE, which no
  longer carries z transposes, so they overlap the SWDGE casts.
- z^2 stats: direct tensor_reduce per 32-j chunk (DVE 2x modes) with
  squares split ScalarE/DVE; no halving tree.
- Bias scale (rsqrt mult) + additive mask ride the idle gpsimd/Pool
  engine.
- v/g projections run in z-phase PE slack instead of the tail.
- Tail unchanged in structure: per 4-head batch qk (PE) -> fused bias
  add (DVE) -> exp with accum denominator (ScalarE) -> normalize ->
  batched xbar transpose (DMA idle in tail) -> attn@v -> out proj.
Rel err ~7e-3 (gate 2e-2), same numeric profile as baseline (bf16
s/weights/B).
"""

import os
from contextlib import ExitStack

import numpy as np

import concourse.bass as bass
import concourse.bacc as bacc
import concourse.tile as tile
import concourse.mybir as mybir
from concourse.masks import make_identity

S = 1024
CS = 384
CZ = 128
D = 32
H = 12
NCORES = 8
RB = S // NCORES  # 128 query rows per core
JT = S // 128     # 8 column tiles
CKS = CS // 128   # 3 contraction chunks of s-dim
EPS = 1e-5
INVD = 1.0 / np.sqrt(D)

F32 = mybir.dt.float32
BF16 = mybir.dt.bfloat16
F16 = mybir.dt.float16
I32 = mybir.dt.int32
AF = mybir.ActivationFunctionType
OP = mybir.AluOpType

BI = 32   # bias psum batch (32*12*4B = 1.5KB <= bank)
SB = 16   # stats chunk (j per square/reduce pass)


def _mm(nc, out, lhsT, rhs, start, stop, **kw):
    nc.tensor.matmul(out, lhsT, rhs, start=start, stop=stop, **kw)


def build(nc):
    s_full = nc.dram_tensor("s", [S, CS], F32, kind="ExternalInput").ap()
    s_loc = nc.dram_tensor("s_loc", [RB, CS], F32, kind="ExternalInput").ap()
    z_d = nc.dram_tensor("z", [RB, S, CZ], F32, kind="ExternalInput").ap()
    zm_d = nc.dram_tensor("z_mask", [RB, S], I32, kind="ExternalInput").ap()
    ws_d = nc.dram_tensor("w_s", [CS], F32, kind="ExternalInput").ap()
    wz_d = nc.dram_tensor("w_z", [CZ], F32, kind="ExternalInput").ap()
    Wz_d = nc.dram_tensor("Wz", [CZ, H], F32, kind="ExternalInput").ap()
    Wq_d = nc.dram_tensor("Wq", [CS, CS], F32, kind="ExternalInput").ap()
    Wk_d = nc.dram_tensor("Wk", [CS, CS], F32, kind="ExternalInput").ap()
    Wv_d = nc.dram_tensor("Wv", [CS, CS], F32, kind="ExternalInput").ap()
    Wg_d = nc.dram_tensor("Wg", [CS, CS], F32, kind="ExternalInput").ap()
    bg_d = nc.dram_tensor("bg", [CS], F32, kind="ExternalInput").ap()
    Wo_d = nc.dram_tensor("Wo", [CS, CS], F32, kind="ExternalInput").ap()
    bo_d = nc.dram_tensor("bo", [CS], F32, kind="ExternalInput").ap()
    out_d = nc.dram_tensor("out", [RB, CS], F32, kind="ExternalOutput").ap()

    with tile.TileContext(nc) as tc, ExitStack() as ctx:
        sg = ctx.enter_context(tc.tile_pool(name="singles", bufs=1))
        # preamble + tail-persistent pools outlive the z pools (LIFO)
        pctx = ExitStack()
        pre = pctx.enter_context(tc.tile_pool(name="pre", bufs=1))
        pp = pctx.enter_context(tc.tile_pool(name="pre_ps", bufs=2,
                                             space="PSUM"))
        tlw = pctx.enter_context(tc.tile_pool(name="tailw", bufs=1))
        zctx = ExitStack()
        znp = zctx.enter_context(tc.tile_pool(name="znat", bufs=4))
        ztp = zctx.enter_context(tc.tile_pool(name="znT", bufs=1))
        sqp = zctx.enter_context(tc.tile_pool(name="sqp", bufs=2))
        trp = zctx.enter_context(tc.tile_pool(name="trp", bufs=2))
        msp = zctx.enter_context(tc.tile_pool(name="msp", bufs=2))
        wpl = zctx.enter_context(tc.tile_pool(name="wpool", bufs=1))
        tpp = zctx.enter_context(tc.tile_pool(name="tr_ps", bufs=2,
                                              space="PSUM"))
        bpp = zctx.enter_context(tc.tile_pool(name="bias_ps", bufs=2,
                                              space="PSUM"))

        zn_tiles = {}

        def issue_cast(jt, half=None):
            # half-tile casts: znI slots recycle at 64-j granularity so a
            # late stats chain on one half doesn't stall the cast FIFO
            if half is None:
                issue_cast(jt, 0)
                issue_cast(jt, 1)
                return
            znI = znp.tile([128, 64, CZ], BF16, tag="zn", name="znI")
            nc.gpsimd.dma_start(
                out=znI, in_=z_d[:, jt * 128 + half * 64:
                                 jt * 128 + (half + 1) * 64, :])
            zn_tiles[(jt, half)] = znI

        # all aux + casts ride the SWDGE FIFO in explicit order: the aux
        # bytes are on the shared-HBM critical path no matter which ring
        # carries them, and HWDGE transfers starve behind 16-engine SWDGE
        # bursts, so ordering one FIFO is the robust choice. s first
        # (pre_s1 fills the cast(0) window), weights spread between casts.
        s8_f = sg.tile([128, JT, CS], F32)
        nc.gpsimd.dma_start(
            out=s8_f, in_=s_full.rearrange("(t p) c -> p t c", p=128))
        sl_f = sg.tile([128, CS], F32)
        nc.gpsimd.dma_start(out=sl_f, in_=s_loc)
        issue_cast(0)

        # ---------- consts + tiny aux (SWDGE, before cast(1)) ----------
        ident_b = sg.tile([128, 128], BF16)
        make_identity(nc, ident_b)
        ones1 = sg.tile([1, 128], F32)
        nc.vector.memset(ones1, 1.0)
        eps_t = sg.tile([128, 1], F32)
        nc.vector.memset(eps_t, EPS)

        Wz_sb = sg.tile([128, H], F32)
        nc.gpsimd.dma_start(out=Wz_sb, in_=Wz_d)
        wzv_sb = sg.tile([128, 1], F32)
        nc.gpsimd.dma_start(out=wzv_sb, in_=wz_d.rearrange("(p o) -> p o", o=1))
        ws_sb = sg.tile([128, CKS], F32)
        nc.gpsimd.dma_start(out=ws_sb, in_=ws_d.rearrange("(k p) -> p k", p=128))
        bg_sb = sg.tile([1, CS], F32)
        nc.gpsimd.dma_start(out=bg_sb, in_=bg_d.rearrange("(o c) -> o c", o=1))
        bo_sb = sg.tile([1, CS], F32)
        nc.gpsimd.dma_start(out=bo_sb, in_=bo_d.rearrange("(o c) -> o c", o=1))

        nc.vector.tensor_scalar_mul(Wz_sb, Wz_sb, wzv_sb)
        Wz_bf = sg.tile([128, H], BF16)
        nc.vector.tensor_copy(out=Wz_bf, in_=Wz_sb)

        # persisted bias tiles [i, jt, j, h] + additive attention mask
        B_all = sg.tile([128, JT, 128, H], BF16)
        maskneg = sg.tile([128, S], BF16)
        mi_sb = sg.tile([128, S], I32)

        def mask_add_park(jt, eng):
            B_loc = B_all[:, jt, :, :]
            mslice = bass.AP(
                tensor=maskneg.tensor, offset=maskneg.offset + jt * 128,
                ap=[maskneg.ap[0], [1, 128], [0, H]])
            eng.tensor_tensor(out=B_loc, in0=B_loc, in1=mslice, op=OP.add)

        # weight loads: SWDGE cast-DMAs (f32 -> bf16 in the DMA), placed
        # in the FIFO between z casts by call order
        def load_w(dram, pool, name="wb"):
            wb = pool.tile([128, CKS, CS], BF16, tag=name, name=name)
            nc.gpsimd.dma_start(
                out=wb, in_=dram.rearrange("(k p) c -> p k c", p=128))
            return wb

        # ---------- per-jt z pipeline ----------
        # Measured engine rates drive this split: ScalarE Square is the
        # only fast squaring path (DVE self-mult pays a 2-read penalty),
        # DVE tree adds on distinct slices get ~2x, f32-out reduces are
        # stuck at 1x, PSUM drains are cheapest on DVE (2x), and Pool
        # cannot touch PSUM at all (its sole heavy job: one square chunk).
        def z_step(jt):
            B_loc = B_all[:, jt, :, :]
            zh = [zn_tiles.pop((jt, 0)), zn_tiles.pop((jt, 1))]
            # stats first in program order: squares have no PE dependency,
            # so they can't get stuck behind the engines' drain waits.
            # 8 chunks of 16 j, sq slots 4 deep to decouple the chain.
            msI = msp.tile([128, 128], F32, tag="msI", name="msI")
            for q in range(8):
                sq = sqp.tile([128, SB, CZ], BF16, tag="sq", name="sq")
                src = zh[q // 4][:, bass.ts(q % 4, SB), :]
                if q >= 6:
                    nc.gpsimd.tensor_tensor(out=sq, in0=src, in1=src,
                                            op=OP.mult)
                else:
                    nc.scalar.square(out=sq, in_=src)
                # tree stages write SEPARATE tiles: 3-AP-same-tile in-place
                # ops fall off the DVE fast path (~4x slower, measured)
                tA = trp.tile([128, SB, 64], BF16, tag="tA", name="tA")
                tA_eng = nc.gpsimd if q % 2 == 0 else nc.vector
                tA_eng.tensor_tensor(
                    out=tA, in0=sq[:, :, 0:64], in1=sq[:, :, 64:128],
                    op=OP.add)
                tB = trp.tile([128, SB, 32], BF16, tag="tB", name="tB")
                nc.vector.tensor_tensor(
                    out=tB, in0=tA[:, :, 0:32], in1=tA[:, :, 32:64],
                    op=OP.add)
                nc.vector.tensor_reduce(
                    out=msI[:, bass.ts(q, SB)], in_=tB,
                    axis=mybir.AxisListType.X, op=OP.add)
                if q % 2 == 1:
                    # rsqrt per 32-j pair so bias scaling never waits the
                    # whole stats pass
                    mpair = msI[:, (q - 1) * SB:(q + 1) * SB]
                    nc.scalar.activation(out=mpair, in_=mpair, func=AF.Sqrt,
                                         bias=eps_t, scale=float(1.0 / CZ))
                    nc.vector.reciprocal(out=mpair, in_=mpair)
            # PE transposes per j-slice -> PSUM (bf16), drain to zt
            zt = ztp.tile([128, 128, 128], BF16, tag="zt", name="zt")
            for bg in range(8):
                ps = tpp.tile([128, 16, 128], BF16, tag="tps", name="tps")
                for jj in range(16):
                    _mm(nc, ps[:, jj, :], zh[bg // 4][:, (bg % 4) * 16 + jj, :],
                        ident_b, True, True, is_transpose=True)
                dst = zt[:, bg * 16:(bg + 1) * 16, :]
                if bg % 2 == 1:
                    nc.scalar.copy(out=dst, in_=ps)
                else:
                    nc.vector.tensor_copy(out=dst, in_=ps)
            # cast(jt+2): the znI-slot waits ride the Pool engine; h1 comes
            # after Pool's own h1 squares in Pool program order (q6/q7)
            if jt + 2 < JT:
                issue_cast(jt + 2, 0)
                issue_cast(jt + 2, 1)
            # bias matmuls: B[i, j, h] = zt[:, j, :].T @ Wz
            for b in range(RB // BI):
                j0 = b * BI
                b_ps = bpp.tile([128, BI, H], F32, tag="bps", name="b_ps")
                for jj in range(BI):
                    _mm(nc, b_ps[:, jj, :], zt[:, j0 + jj, :], Wz_bf,
                        jj == 0, jj == BI - 1)
                rs_b = bass.AP(
                    tensor=msI.tensor,
                    offset=msI.offset + j0,
                    ap=[msI.ap[0], [1, BI], [0, H]])
                nc.vector.tensor_tensor(
                    out=B_loc[:, j0:j0 + BI, :], in0=b_ps, in1=rs_b,
                    op=OP.mult)
            return B_loc

        # ---------- preamble (fills the cast(0)/cast(1) window) ----------
        s_rT = pre.tile([128, CKS, S], BF16)     # [c, k, i]
        s_rTl = pre.tile([128, CKS, 128], BF16)  # [c, k, local i]
        kT = pre.tile([128, CKS, S], BF16)       # [hd_in_chunk, chunk, j]
        qT = pre.tile([128, CKS, 128], BF16)     # [hd_in_chunk, chunk, i_loc]
        _ncopy = [0]

        def drain(dst, psrc):
            if _ncopy[0] % 2 == 0:
                nc.vector.tensor_copy(out=dst, in_=psrc)
            else:
                nc.scalar.copy(out=dst, in_=psrc)
            _ncopy[0] += 1

        def pre_s1():
            with tc.tile_pool(name="pre_tmp", bufs=2) as pt:
                def norm_rows(ap, out_bf):
                    nsq = pt.tile([128, CS], BF16, tag="nsq")
                    msum = pt.tile([128, 1], F32, tag="nms")
                    nc.scalar.activation(out=nsq, in_=ap, func=AF.Square,
                                         scale=float(1.0 / np.sqrt(CS)),
                                         accum_out=msum)
                    nc.scalar.activation(out=msum, in_=msum, func=AF.Sqrt,
                                         bias=eps_t, scale=1.0)
                    nc.vector.reciprocal(out=msum, in_=msum)
                    nc.vector.scalar_tensor_tensor(
                        out=out_bf, in0=ap, scalar=1.0,
                        in1=bass.AP(tensor=msum.tensor, offset=msum.offset,
                                    ap=[msum.ap[0], [0, CS]]),
                        op0=OP.mult, op1=OP.mult)

                def tr128(dst, src):
                    # psum slot shared with tag "big" (same 2KB bank)
                    ps = pp.tile([128, 1024], BF16, tag="big")
                    _mm(nc, ps[:, 0:128], src, ident_b,
                        True, True, is_transpose=True)
                    drain(dst, ps[:, 0:128])

                for t in range(JT):
                    s_rb = pt.tile([128, CS], BF16, tag="s_rb")
                    norm_rows(s8_f[:, t, :], s_rb)
                    for k in range(CKS):
                        tr128(s_rT[:, k, bass.ts(t, 128)],
                              s_rb[:, bass.ts(k, 128)])
                s_rlb = pt.tile([128, CS], BF16, tag="s_rb")
                norm_rows(sl_f, s_rlb)
                for k in range(CKS):
                    tr128(s_rTl[:, k, :], s_rlb[:, bass.ts(k, 128)])
            for k in range(CKS):
                nc.vector.tensor_scalar_mul(
                    s_rT[:, k, :], s_rT[:, k, :], ws_sb[:, k:k + 1])
                nc.vector.tensor_scalar_mul(
                    s_rTl[:, k, :], s_rTl[:, k, :], ws_sb[:, k:k + 1])

        wbox = {}

        def pre_qt():
            wb = wbox.pop("wq")
            for k in range(CKS):
                ps = pp.tile([128, 512], F32, tag="big")
                for ck in range(CKS):
                    _mm(nc, ps[:, 0:128], wb[:, ck, bass.ts(k, 128)],
                        s_rTl[:, ck, :], ck == 0, ck == CKS - 1)
                nc.scalar.mul(out=qT[:, k, :], in_=ps[:, 0:128],
                              mul=float(INVD))

        def pre_kt(ks):
            wb = wbox["wk"]
            for k in ks:
                for half in range(2):
                    ps2 = pp.tile([128, 512], F32, tag="big")
                    for ck in range(CKS):
                        _mm(nc, ps2, wb[:, ck, bass.ts(k, 128)],
                            s_rT[:, ck, bass.ts(half, 512)], ck == 0,
                            ck == CKS - 1)
                    drain(kT[:, k, bass.ts(half, 512)], ps2)

        # tail-persistent tensors
        v_sb = tlw.tile([128, JT, CS], BF16)  # [j_in_tile, jt, hd]
        g_sb = tlw.tile([128, CS], F32)
        wo_f = tlw.tile([128, CKS, CS], F32, name="w_Wo")
        oT_sb = tlw.tile([128, CKS, 128], F32)  # [hd_in_chunk, chunk, i]

        def v_and_g():
            wb = wbox["wv"]
            for jc in range(JT):
                ps2 = pp.tile([128, 512], F32, tag="big")
                for ck in range(CKS):
                    _mm(nc, ps2[:, 0:CS], s_rT[:, ck, bass.ts(jc, 128)],
                        wb[:, ck, :], ck == 0, ck == CKS - 1)
                drain(v_sb[:, jc, :], ps2[:, 0:CS])
            wbg = wbox["wg"]
            ps2 = pp.tile([128, 512], F32, tag="big")
            for ck in range(CKS):
                _mm(nc, ps2[:, 0:CS], s_rTl[:, ck, :], wbg[:, ck, :],
                    ck == 0, False)
            _mm(nc, ps2[:, 0:CS], ones1, bg_sb, False, True)
            nc.scalar.copy(out=g_sb, in_=ps2[:, 0:CS])

        # SWDGE FIFO continues: wq, wk, cast(1), wv, wg, wo, mask, casts
        wbox["wq"] = load_w(Wq_d, wpl, name="wq")
        wbox["wk"] = load_w(Wk_d, wpl, name="wk")
        issue_cast(1)
        wbox["wv"] = load_w(Wv_d, wpl, name="wv")
        wbox["wg"] = load_w(Wg_d, wpl, name="wg")
        nc.gpsimd.dma_start(
            out=wo_f, in_=Wo_d.rearrange("(k p) c -> p k c", p=128))
        nc.gpsimd.dma_start(out=mi_sb, in_=zm_d)
        nc.vector.tensor_scalar(
            out=maskneg, in0=mi_sb, scalar1=0, scalar2=-30000.0,
            op0=OP.is_equal, op1=OP.mult)

        # preamble compute while cast(0)/cast(1) stream in
        pre_s1()
        pre_qt()

        # ---------- z steps; kT built in early-step slack, mask parks
        # (Pool) once the mask has landed ----------
        stages = {0: lambda: pre_kt([0]), 1: lambda: pre_kt([1]),
                  2: lambda: pre_kt([2]),
                  4: lambda: [mask_add_park(t, nc.gpsimd) for t in (0, 1, 2)],
                  5: v_and_g,
                  6: lambda: [mask_add_park(t, nc.gpsimd) for t in (3, 4, 5, 6)]}
        for jt in range(JT):
            z_step(jt)
            if jt in stages:
                stages[jt]()
        mask_add_park(7, nc.gpsimd)
        zctx.close()

        # ---------- scores + attention tail ----------
        # bias-add split by head parity: even heads accumulate B into the
        # qk PSUM on the PE (exp reads PSUM), odd heads do a DVE STT into
        # an f16 scores tile (exp reads that) — balances PE vs DVE, both
        # ~50% loaded in the tail.
        HB = 4   # heads per transpose batch
        with tc.tile_pool(name="att_sb", bufs=2) as asb, \
             tc.tile_pool(name="attT_sb", bufs=2) as atsb, \
             tc.tile_pool(name="att16", bufs=2) as a16p, \
             tc.tile_pool(name="den_sb", bufs=2) as dsb, \
             tc.tile_pool(name="qk_ps", bufs=2, space="PSUM") as qkp, \
             tc.tile_pool(name="o_ps", bufs=2, space="PSUM") as opp:
            for batch in range(H // HB):
                attb = asb.tile([128, HB, JT, 128], BF16, tag="attb",
                                name="attb")
                for hh in range(HB):
                    h = batch * HB + hh
                    ck, hp = divmod(h, 4)
                    qk = qkp.tile([128, S], F32, tag="qk", name="qk")
                    pe_bias = True
                    for half in range(2):
                        _mm(nc, qk[:, bass.ts(half, 512)],
                            qT[bass.ts(hp, 32), ck, :],
                            kT[bass.ts(hp, 32), ck, bass.ts(half, 512)],
                            True, not pe_bias, tile_position=(32 * hp, 0))
                        if pe_bias:
                            b_half = bass.AP(
                                tensor=B_all.tensor,
                                offset=B_all.offset + h + half * 4 * (128 * H),
                                ap=[B_all.ap[0], [128 * H, 4], [H, 128]])
                            _mm(nc, qk[:, bass.ts(half, 512)], ident_b,
                                b_half, False, True)
                    att = attb[:, hh, :, :]
                    den = dsb.tile([128, 1], F32, tag="den", name="den")
                    if pe_bias:
                        exp_src = qk.rearrange("p (t j) -> p t j", j=128)
                    else:
                        att16 = a16p.tile([128, S], F16, tag="a16",
                                          name="att16")
                        b_slice = bass.AP(
                            tensor=B_all.tensor,
                            offset=B_all.offset + h,
                            ap=[B_all.ap[0], [128 * H, JT], [H, 128]])
                        qk3 = bass.AP(
                            tensor=qk.tensor, offset=qk.offset,
                            ap=[qk.ap[0], [128, JT], [1, 128]])
                        a163 = bass.AP(
                            tensor=att16.tensor, offset=att16.offset,
                            ap=[att16.ap[0], [128, JT], [1, 128]])
                        nc.vector.scalar_tensor_tensor(
                            out=a163, in0=qk3, scalar=1.0,
                            in1=b_slice, op0=OP.mult, op1=OP.add)
                        exp_src = att16.rearrange("p (t j) -> p t j", j=128)
                    nc.scalar.activation(out=att, in_=exp_src, func=AF.Exp,
                                         accum_out=den)
                    nc.vector.reciprocal(out=den, in_=den)
                    nc.vector.tensor_scalar_mul(att, att, den)
                atT = atsb.tile([128, HB, JT, 128], BF16, tag="atT",
                                name="atT")
                nc.sync.dma_start(out=atT, in_=attb, transpose=True)
                for hh in range(HB):
                    h = batch * HB + hh
                    o_ps = opp.tile([32, 128], F32, tag="o", name="o_ps")
                    for jc in range(JT):
                        _mm(nc, o_ps, v_sb[:, jc, bass.ts(h, 32)],
                            atT[:, hh, jc, :], jc == 0, jc == JT - 1)
                    ck, hp = divmod(h, 4)
                    if h % 2 == 0:
                        nc.vector.tensor_copy(
                            out=oT_sb[bass.ts(hp, 32), ck, :], in_=o_ps)
                    else:
                        nc.scalar.copy(out=oT_sb[bass.ts(hp, 32), ck, :],
                                       in_=o_ps)

            # ---------- output ----------
            fin = pp.tile([128, 512], F32, tag="big")
            for k in range(CKS):
                _mm(nc, fin[:, 0:CS], oT_sb[:, k, :], wo_f[:, k, :],
                    k == 0, False)
            _mm(nc, fin[:, 0:CS], ones1, bo_sb, False, True)
            out_sb = tlw.tile([128, CS], F32)
            nc.vector.tensor_tensor(out=out_sb, in0=fin[:, 0:CS], in1=g_sb,
                                    op=OP.mult)
            nc.sync.dma_start(out=out_d, in_=out_sb)
        pctx.close()

    nc.compile()
    return nc


_NC_CACHE = None


def _get_nc():
    global _NC_CACHE
    if _NC_CACHE is None:
        nc = bacc.Bacc("TRN2", target_bir_lowering=False, debug=False,
                       enable_asserts=False)
        _NC_CACHE = build(nc)
    return _NC_CACHE


def make_in_maps(s, z, z_mask, w_s, w_z, Wz, Wq, Wk, Wv, Wg, bg, Wo, bo):
    f = lambda a: np.ascontiguousarray(np.asarray(a), dtype=np.float32)
    s = f(s)
    shared = dict(s=s, w_s=f(w_s), w_z=f(w_z), Wz=f(Wz), Wq=f(Wq), Wk=f(Wk),
                  Wv=f(Wv), Wg=f(Wg), bg=f(bg), Wo=f(Wo), bo=f(bo))
    zmask = np.ascontiguousarray(np.asarray(z_mask), dtype=np.int32)
    z = f(z)
    in_maps = []
    for c in range(NCORES):
        r0, r1 = c * RB, (c + 1) * RB
        m = dict(shared)
        m["s_loc"] = np.ascontiguousarray(s[r0:r1])
        m["z"] = np.ascontiguousarray(z[r0:r1])
        m["z_mask"] = np.ascontiguousarray(zmask[r0:r1])
        in_maps.append(m)
    return in_maps


def kernel(**inputs):
    from concourse import bass_utils
    nc = _get_nc()
    in_maps = make_in_maps(**inputs)
    res = bass_utils.run_bass_kernel_spmd(nc, in_maps, core_ids=list(range(NCORES)))
    out = np.concatenate([res.results[c]["out"] for c in range(NCORES)], axis=0)
    return out.astype(np.float32)


# revision 37
# speedup vs baseline: 1.2209x; 1.0607x over previous
"""AttentionWithPairBias distributed Trainium2 kernel (8 NeuronCores).

Sequence-parallel: core c owns query rows i in [128c, 128(c+1)).
Per core: z shard [128, 1024, 128] f32 (64MB -> the memory roofline),
s and weights replicated. No collectives.

v2 (trace-driven rewrite of the 453-556us baseline):
- The baseline serialized the f32->bf16 z cast DMA (HBM-bound,
  ~22us/jt) with the SBUF->SBUF xbar transpose (~19us/jt) because both
  use all 16 DMA engines and the xbar mode switch drains in-flight
  DMAs -> ~45us/jt. Fix: transpose z on the PE instead (128
  is_transpose matmuls per jt, ~7us at 2.4GHz) with PSUM->SBUF drains
  split across DVE/ScalarE/Pool. The DMA engines then carry ONLY the
  casts, back-to-back at the HBM roofline (~22.3us/jt).
- All aux loads (s, weights, mask) ride the sync-ring HWDGE, which no
  longer carries z transposes, so they overlap the SWDGE casts.
- z^2 stats: direct tensor_reduce per 32-j chunk (DVE 2x modes) with
  squares split ScalarE/DVE; no halving tree.
- Bias scale (rsqrt mult) + additive mask ride the idle gpsimd/Pool
  engine.
- v/g projections run in z-phase PE slack instead of the tail.
- Tail unchanged in structure: per 4-head batch qk (PE) -> fused bias
  add (DVE) -> exp with accum denominator (ScalarE) -> normalize ->
  batched xbar transpose (DMA idle in tail) -> attn@v -> out proj.
Rel err ~7e-3 (gate 2e-2), same numeric profile as baseline (bf16
s/weights/B).
"""

import os
from contextlib import ExitStack

import numpy as np

import concourse.bass as bass
import concourse.bacc as bacc
import concourse.tile as tile
import concourse.mybir as mybir
from concourse.masks import make_identity

S = 1024
CS = 384
CZ = 128
D = 32
H = 12
NCORES = 8
RB = S // NCORES  # 128 query rows per core
JT = S // 128     # 8 column tiles
CKS = CS // 128   # 3 contraction chunks of s-dim
EPS = 1e-5
INVD = 1.0 / np.sqrt(D)

F32 = mybir.dt.float32
BF16 = mybir.dt.bfloat16
F16 = mybir.dt.float16
I32 = mybir.dt.int32
AF = mybir.ActivationFunctionType
OP = mybir.AluOpType

BI = 32   # bias psum batch (32*12*4B = 1.5KB <= bank)
SB = 16   # stats chunk (j per square/reduce pass)


def _mm(nc, out, lhsT, rhs, start, stop, **kw):
    nc.tensor.matmul(out, lhsT, rhs, start=start, stop=stop, **kw)


def build(nc):
    s_full = nc.dram_tensor("s", [S, CS], F32, kind="ExternalInput").ap()
    s_loc = nc.dram_tensor("s_loc", [RB, CS], F32, kind="ExternalInput").ap()
    z_d = nc.dram_tensor("z", [RB, S, CZ], F32, kind="ExternalInput").ap()
    zm_d = nc.dram_tensor("z_mask", [RB, S], I32, kind="ExternalInput").ap()
    ws_d = nc.dram_tensor("w_s", [CS], F32, kind="ExternalInput").ap()
    wz_d = nc.dram_tensor("w_z", [CZ], F32, kind="ExternalInput").ap()
    Wz_d = nc.dram_tensor("Wz", [CZ, H], F32, kind="ExternalInput").ap()
    Wq_d = nc.dram_tensor("Wq", [CS, CS], F32, kind="ExternalInput").ap()
    Wk_d = nc.dram_tensor("Wk", [CS, CS], F32, kind="ExternalInput").ap()
    Wv_d = nc.dram_tensor("Wv", [CS, CS], F32, kind="ExternalInput").ap()
    Wg_d = nc.dram_tensor("Wg", [CS, CS], F32, kind="ExternalInput").ap()
    bg_d = nc.dram_tensor("bg", [CS], F32, kind="ExternalInput").ap()
    Wo_d = nc.dram_tensor("Wo", [CS, CS], F32, kind="ExternalInput").ap()
    bo_d = nc.dram_tensor("bo", [CS], F32, kind="ExternalInput").ap()
    out_d = nc.dram_tensor("out", [RB, CS], F32, kind="ExternalOutput").ap()

    with tile.TileContext(nc) as tc, ExitStack() as ctx:
        sg = ctx.enter_context(tc.tile_pool(name="singles", bufs=1))
        # preamble + tail-persistent pools outlive the z pools (LIFO)
        pctx = ExitStack()
        pre = pctx.enter_context(tc.tile_pool(name="pre", bufs=1))
        pp = pctx.enter_context(tc.tile_pool(name="pre_ps", bufs=1,
                                             space="PSUM"))
        tlw = pctx.enter_context(tc.tile_pool(name="tailw", bufs=1))
        zctx = ExitStack()
        znp = zctx.enter_context(tc.tile_pool(name="znat", bufs=4))
        ztp = zctx.enter_context(tc.tile_pool(name="znT", bufs=1))
        sqp = zctx.enter_context(tc.tile_pool(name="sqp", bufs=2))
        trp = zctx.enter_context(tc.tile_pool(name="trp", bufs=2))
        msp = zctx.enter_context(tc.tile_pool(name="msp", bufs=2))
        wpl = zctx.enter_context(tc.tile_pool(name="wpool", bufs=1))
        tpp = zctx.enter_context(tc.tile_pool(name="tr_ps", bufs=2,
                                              space="PSUM"))
        bpp = zctx.enter_context(tc.tile_pool(name="bias_ps", bufs=3,
                                              space="PSUM"))

        zn_tiles = {}

        def issue_cast(jt, half=None):
            # half-tile casts: znI slots recycle at 64-j granularity so a
            # late stats chain on one half doesn't stall the cast FIFO
            if half is None:
                issue_cast(jt, 0)
                issue_cast(jt, 1)
                return
            znI = znp.tile([128, 64, CZ], BF16, tag="zn", name="znI")
            nc.gpsimd.dma_start(
                out=znI, in_=z_d[:, jt * 128 + half * 64:
                                 jt * 128 + (half + 1) * 64, :])
            zn_tiles[(jt, half)] = znI

        # all aux + casts ride the SWDGE FIFO in explicit order: the aux
        # bytes are on the shared-HBM critical path no matter which ring
        # carries them, and HWDGE transfers starve behind 16-engine SWDGE
        # bursts, so ordering one FIFO is the robust choice. s first
        # (pre_s1 fills the cast(0) window), weights spread between casts.
        s8_f = sg.tile([128, JT, CS], F32)
        nc.gpsimd.dma_start(
            out=s8_f, in_=s_full.rearrange("(t p) c -> p t c", p=128))
        sl_f = sg.tile([128, CS], F32)
        nc.gpsimd.dma_start(out=sl_f, in_=s_loc)
        issue_cast(0)

        # ---------- consts + tiny aux (SWDGE, before cast(1)) ----------
        ident_b = sg.tile([128, 128], BF16)
        make_identity(nc, ident_b)
        ones1 = sg.tile([1, 128], F32)
        nc.vector.memset(ones1, 1.0)
        eps_t = sg.tile([128, 1], F32)
        nc.vector.memset(eps_t, EPS)

        Wz_sb = sg.tile([128, H], F32)
        nc.gpsimd.dma_start(out=Wz_sb, in_=Wz_d)
        wzv_sb = sg.tile([128, 1], F32)
        nc.gpsimd.dma_start(out=wzv_sb, in_=wz_d.rearrange("(p o) -> p o", o=1))
        ws_sb = sg.tile([128, CKS], F32)
        nc.gpsimd.dma_start(out=ws_sb, in_=ws_d.rearrange("(k p) -> p k", p=128))
        bg_sb = sg.tile([1, CS], F32)
        nc.gpsimd.dma_start(out=bg_sb, in_=bg_d.rearrange("(o c) -> o c", o=1))
        bo_sb = sg.tile([1, CS], F32)
        nc.gpsimd.dma_start(out=bo_sb, in_=bo_d.rearrange("(o c) -> o c", o=1))

        nc.vector.tensor_scalar_mul(Wz_sb, Wz_sb, wzv_sb)
        Wz_bf = sg.tile([128, H], BF16)
        nc.vector.tensor_copy(out=Wz_bf, in_=Wz_sb)

        # persisted bias tiles [i, jt, j, h] + additive attention mask
        B_all = sg.tile([128, JT, 128, H], BF16)
        maskneg = sg.tile([128, S], BF16)
        mi_sb = sg.tile([128, S], I32)

        def mask_add_park(jt, eng):
            B_loc = B_all[:, jt, :, :]
            mslice = bass.AP(
                tensor=maskneg.tensor, offset=maskneg.offset + jt * 128,
                ap=[maskneg.ap[0], [1, 128], [0, H]])
            eng.tensor_tensor(out=B_loc, in0=B_loc, in1=mslice, op=OP.add)

        # weight loads: SWDGE cast-DMAs (f32 -> bf16 in the DMA), placed
        # in the FIFO between z casts by call order
        def load_w(dram, pool, name="wb"):
            wb = pool.tile([128, CKS, CS], BF16, tag=name, name=name)
            nc.gpsimd.dma_start(
                out=wb, in_=dram.rearrange("(k p) c -> p k c", p=128))
            return wb

        # ---------- per-jt z pipeline ----------
        # Measured engine rates drive this split: ScalarE Square is the
        # only fast squaring path (DVE self-mult pays a 2-read penalty),
        # DVE tree adds on distinct slices get ~2x, f32-out reduces are
        # stuck at 1x, PSUM drains are cheapest on DVE (2x), and Pool
        # cannot touch PSUM at all (its sole heavy job: one square chunk).
        def z_step(jt):
            B_loc = B_all[:, jt, :, :]
            zh = [zn_tiles.pop((jt, 0)), zn_tiles.pop((jt, 1))]
            # stats first in program order: squares have no PE dependency,
            # so they can't get stuck behind the engines' drain waits.
            # 8 chunks of 16 j, sq slots 4 deep to decouple the chain.
            msI = msp.tile([128, 128], F32, tag="msI", name="msI")
            for q in range(8):
                sq = sqp.tile([128, SB, CZ], BF16, tag="sq", name="sq")
                src = zh[q // 4][:, bass.ts(q % 4, SB), :]
                if q >= 6:
                    nc.gpsimd.tensor_tensor(out=sq, in0=src, in1=src,
                                            op=OP.mult)
                else:
                    nc.scalar.square(out=sq, in_=src)
                # tree stages write SEPARATE tiles: 3-AP-same-tile in-place
                # ops fall off the DVE fast path (~4x slower, measured)
                tA = trp.tile([128, SB, 64], BF16, tag="tA", name="tA")
                tA_eng = nc.gpsimd if q % 2 == 0 else nc.vector
                tA_eng.tensor_tensor(
                    out=tA, in0=sq[:, :, 0:64], in1=sq[:, :, 64:128],
                    op=OP.add)
                tB = trp.tile([128, SB, 32], BF16, tag="tB", name="tB")
                nc.vector.tensor_tensor(
                    out=tB, in0=tA[:, :, 0:32], in1=tA[:, :, 32:64],
                    op=OP.add)
                nc.vector.tensor_reduce(
                    out=msI[:, bass.ts(q, SB)], in_=tB,
                    axis=mybir.AxisListType.X, op=OP.add)
                if q % 2 == 1:
                    # rsqrt per 32-j pair so bias scaling never waits the
                    # whole stats pass
                    mpair = msI[:, (q - 1) * SB:(q + 1) * SB]
                    nc.scalar.activation(out=mpair, in_=mpair, func=AF.Sqrt,
                                         bias=eps_t, scale=float(1.0 / CZ))
                    nc.vector.reciprocal(out=mpair, in_=mpair)
            # PE transposes per j-slice -> PSUM (bf16), drain to zt
            zt = ztp.tile([128, 128, 128], BF16, tag="zt", name="zt")
            for bg in range(8):
                ps = tpp.tile([128, 16, 128], BF16, tag="tps", name="tps")
                for jj in range(16):
                    _mm(nc, ps[:, jj, :], zh[bg // 4][:, (bg % 4) * 16 + jj, :],
                        ident_b, True, True, is_transpose=True)
                dst = zt[:, bg * 16:(bg + 1) * 16, :]
                if bg % 2 == 1:
                    nc.scalar.copy(out=dst, in_=ps)
                else:
                    nc.vector.tensor_copy(out=dst, in_=ps)
            # cast(jt+2): the znI-slot waits ride the Pool engine; h1 comes
            # after Pool's own h1 squares in Pool program order (q6/q7)
            if jt + 2 < JT:
                issue_cast(jt + 2, 0)
                issue_cast(jt + 2, 1)
            # bias matmuls: B[i, j, h] = zt[:, j, :].T @ Wz
            for b in range(RB // BI):
                j0 = b * BI
                b_ps = bpp.tile([128, BI, H], F32, tag="bps", name="b_ps")
                for jj in range(BI):
                    _mm(nc, b_ps[:, jj, :], zt[:, j0 + jj, :], Wz_bf,
                        jj == 0, jj == BI - 1)
                rs_b = bass.AP(
                    tensor=msI.tensor,
                    offset=msI.offset + j0,
                    ap=[msI.ap[0], [1, BI], [0, H]])
                nc.vector.tensor_tensor(
                    out=B_loc[:, j0:j0 + BI, :], in0=b_ps, in1=rs_b,
                    op=OP.mult)
            return B_loc

        # ---------- preamble (fills the cast(0)/cast(1) window) ----------
        s_rT = pre.tile([128, CKS, S], BF16)     # [c, k, i]
        s_rTl = pre.tile([128, CKS, 128], BF16)  # [c, k, local i]
        kT = pre.tile([128, CKS, S], BF16)       # [hd_in_chunk, chunk, j]
        qT = pre.tile([128, CKS, 128], BF16)     # [hd_in_chunk, chunk, i_loc]
        _ncopy = [0]

        def drain(dst, psrc):
            if _ncopy[0] % 2 == 0:
                nc.vector.tensor_copy(out=dst, in_=psrc)
            else:
                nc.scalar.copy(out=dst, in_=psrc)
            _ncopy[0] += 1

        def pre_s1():
            with tc.tile_pool(name="pre_tmp", bufs=2) as pt:
                def norm_rows(ap, out_bf):
                    nsq = pt.tile([128, CS], BF16, tag="nsq")
                    msum = pt.tile([128, 1], F32, tag="nms")
                    nc.scalar.activation(out=nsq, in_=ap, func=AF.Square,
                                         scale=float(1.0 / np.sqrt(CS)),
                                         accum_out=msum)
                    nc.scalar.activation(out=msum, in_=msum, func=AF.Sqrt,
                                         bias=eps_t, scale=1.0)
                    nc.vector.reciprocal(out=msum, in_=msum)
                    nc.vector.scalar_tensor_tensor(
                        out=out_bf, in0=ap, scalar=1.0,
                        in1=bass.AP(tensor=msum.tensor, offset=msum.offset,
                                    ap=[msum.ap[0], [0, CS]]),
                        op0=OP.mult, op1=OP.mult)

                def tr128(dst, src):
                    # psum slot shared with tag "big" (same 2KB bank)
                    ps = pp.tile([128, 1024], BF16, tag="big")
                    _mm(nc, ps[:, 0:128], src, ident_b,
                        True, True, is_transpose=True)
                    drain(dst, ps[:, 0:128])

                for t in range(JT):
                    s_rb = pt.tile([128, CS], BF16, tag="s_rb")
                    norm_rows(s8_f[:, t, :], s_rb)
                    for k in range(CKS):
                        tr128(s_rT[:, k, bass.ts(t, 128)],
                              s_rb[:, bass.ts(k, 128)])
                s_rlb = pt.tile([128, CS], BF16, tag="s_rb")
                norm_rows(sl_f, s_rlb)
                for k in range(CKS):
                    tr128(s_rTl[:, k, :], s_rlb[:, bass.ts(k, 128)])
            for k in range(CKS):
                nc.vector.tensor_scalar_mul(
                    s_rT[:, k, :], s_rT[:, k, :], ws_sb[:, k:k + 1])
                nc.vector.tensor_scalar_mul(
                    s_rTl[:, k, :], s_rTl[:, k, :], ws_sb[:, k:k + 1])

        wbox = {}

        def pre_qt():
            wb = wbox.pop("wq")
            for k in range(CKS):
                ps = pp.tile([128, 512], F32, tag="big")
                for ck in range(CKS):
                    _mm(nc, ps[:, 0:128], wb[:, ck, bass.ts(k, 128)],
                        s_rTl[:, ck, :], ck == 0, ck == CKS - 1)
                nc.scalar.mul(out=qT[:, k, :], in_=ps[:, 0:128],
                              mul=float(INVD))

        def pre_kt(ks):
            wb = wbox["wk"]
            for k in ks:
                for half in range(2):
                    ps2 = pp.tile([128, 512], F32, tag="big")
                    for ck in range(CKS):
                        _mm(nc, ps2, wb[:, ck, bass.ts(k, 128)],
                            s_rT[:, ck, bass.ts(half, 512)], ck == 0,
                            ck == CKS - 1)
                    drain(kT[:, k, bass.ts(half, 512)], ps2)

        # tail-persistent tensors
        v_sb = tlw.tile([128, JT, CS], BF16)  # [j_in_tile, jt, hd]
        g_sb = tlw.tile([128, CS], F32)
        wo_f = tlw.tile([128, CKS, CS], F32, name="w_Wo")
        oT_sb = tlw.tile([128, CKS, 128], F32)  # [hd_in_chunk, chunk, i]

        def v_and_g():
            wb = wbox["wv"]
            for jc in range(JT):
                ps2 = pp.tile([128, 512], F32, tag="big")
                for ck in range(CKS):
                    _mm(nc, ps2[:, 0:CS], s_rT[:, ck, bass.ts(jc, 128)],
                        wb[:, ck, :], ck == 0, ck == CKS - 1)
                drain(v_sb[:, jc, :], ps2[:, 0:CS])
            wbg = wbox["wg"]
            ps2 = pp.tile([128, 512], F32, tag="big")
            for ck in range(CKS):
                _mm(nc, ps2[:, 0:CS], s_rTl[:, ck, :], wbg[:, ck, :],
                    ck == 0, False)
            _mm(nc, ps2[:, 0:CS], ones1, bg_sb, False, True)
            nc.scalar.copy(out=g_sb, in_=ps2[:, 0:CS])

        # SWDGE FIFO continues: wq, wk, cast(1), wv, wg, wo, mask, casts
        wbox["wq"] = load_w(Wq_d, wpl, name="wq")
        wbox["wk"] = load_w(Wk_d, wpl, name="wk")
        issue_cast(1)
        wbox["wv"] = load_w(Wv_d, wpl, name="wv")
        wbox["wg"] = load_w(Wg_d, wpl, name="wg")
        nc.gpsimd.dma_start(
            out=wo_f, in_=Wo_d.rearrange("(k p) c -> p k c", p=128))
        nc.gpsimd.dma_start(out=mi_sb, in_=zm_d)
        nc.vector.tensor_scalar(
            out=maskneg, in0=mi_sb, scalar1=0, scalar2=-30000.0,
            op0=OP.is_equal, op1=OP.mult)

        # preamble compute while cast(0)/cast(1) stream in
        pre_s1()
        pre_qt()

        # ---------- z steps; kT built in early-step slack, mask parks
        # (Pool) once the mask has landed ----------
        stages = {0: lambda: pre_kt([0]), 1: lambda: pre_kt([1]),
                  2: lambda: pre_kt([2]),
                  4: lambda: [mask_add_park(t, nc.gpsimd) for t in (0, 1, 2)],
                  5: v_and_g,
                  6: lambda: [mask_add_park(t, nc.gpsimd) for t in (3, 4, 5, 6)]}
        for jt in range(JT):
            z_step(jt)
            if jt in stages:
                stages[jt]()
        mask_add_park(7, nc.gpsimd)
        zctx.close()

        # ---------- scores + attention tail ----------
        # bias-add split by head parity: even heads accumulate B into the
        # qk PSUM on the PE (exp reads PSUM), odd heads do a DVE STT into
        # an f16 scores tile (exp reads that) — balances PE vs DVE, both
        # ~50% loaded in the tail.
        HB = 4   # heads per transpose batch
        with tc.tile_pool(name="att_sb", bufs=2) as asb, \
             tc.tile_pool(name="attT_sb", bufs=2) as atsb, \
             tc.tile_pool(name="att16", bufs=2) as a16p, \
             tc.tile_pool(name="den_sb", bufs=2) as dsb, \
             tc.tile_pool(name="qk_ps", bufs=3, space="PSUM") as qkp, \
             tc.tile_pool(name="o_ps", bufs=1, space="PSUM") as opp:
            for batch in range(H // HB):
                attb = asb.tile([128, HB, JT, 128], BF16, tag="attb",
                                name="attb")
                for hh in range(HB):
                    h = batch * HB + hh
                    ck, hp = divmod(h, 4)
                    qk = qkp.tile([128, S], F32, tag="qk", name="qk")
                    pe_bias = True
                    for half in range(2):
                        _mm(nc, qk[:, bass.ts(half, 512)],
                            qT[bass.ts(hp, 32), ck, :],
                            kT[bass.ts(hp, 32), ck, bass.ts(half, 512)],
                            True, not pe_bias, tile_position=(32 * hp, 0))
                        if pe_bias:
                            b_half = bass.AP(
                                tensor=B_all.tensor,
                                offset=B_all.offset + h + half * 4 * (128 * H),
                                ap=[B_all.ap[0], [128 * H, 4], [H, 128]])
                            _mm(nc, qk[:, bass.ts(half, 512)], ident_b,
                                b_half, False, True)
                    att = attb[:, hh, :, :]
                    den = dsb.tile([128, 1], F32, tag="den", name="den")
                    if pe_bias:
                        exp_src = qk.rearrange("p (t j) -> p t j", j=128)
                    else:
                        att16 = a16p.tile([128, S], F16, tag="a16",
                                          name="att16")
                        b_slice = bass.AP(
                            tensor=B_all.tensor,
                            offset=B_all.offset + h,
                            ap=[B_all.ap[0], [128 * H, JT], [H, 128]])
                        qk3 = bass.AP(
                            tensor=qk.tensor, offset=qk.offset,
                            ap=[qk.ap[0], [128, JT], [1, 128]])
                        a163 = bass.AP(
                            tensor=att16.tensor, offset=att16.offset,
                            ap=[att16.ap[0], [128, JT], [1, 128]])
                        nc.vector.scalar_tensor_tensor(
                            out=a163, in0=qk3, scalar=1.0,
                            in1=b_slice, op0=OP.mult, op1=OP.add)
                        exp_src = att16.rearrange("p (t j) -> p t j", j=128)
                    nc.scalar.activation(out=att, in_=exp_src, func=AF.Exp,
                                         accum_out=den)
                    nc.vector.reciprocal(out=den, in_=den)
                    nc.vector.tensor_scalar_mul(att, att, den)
                atT = atsb.tile([128, HB, JT, 128], BF16, tag="atT",
                                name="atT")
                nc.sync.dma_start(out=atT, in_=attb, transpose=True)
                for hh in range(HB):
                    h = batch * HB + hh
                    o_ps = opp.tile([32, 128], F32, tag="o", name="o_ps")
                    for jc in range(JT):
                        _mm(nc, o_ps, v_sb[:, jc, bass.ts(h, 32)],
                            atT[:, hh, jc, :], jc == 0, jc == JT - 1)
                    ck, hp = divmod(h, 4)
                    if h % 2 == 0:
                        nc.vector.tensor_copy(
                            out=oT_sb[bass.ts(hp, 32), ck, :], in_=o_ps)
                    else:
                        nc.scalar.copy(out=oT_sb[bass.ts(hp, 32), ck, :],
                                       in_=o_ps)

            # ---------- output ----------
            fin = pp.tile([128, 512], F32, tag="big")
            for k in range(CKS):
                _mm(nc, fin[:, 0:CS], oT_sb[:, k, :], wo_f[:, k, :],
                    k == 0, False)
            _mm(nc, fin[:, 0:CS], ones1, bo_sb, False, True)
            out_sb = tlw.tile([128, CS], F32)
            nc.vector.tensor_tensor(out=out_sb, in0=fin[:, 0:CS], in1=g_sb,
                                    op=OP.mult)
            nc.sync.dma_start(out=out_d, in_=out_sb)
        pctx.close()

    nc.compile()
    return nc


_NC_CACHE = None


def _get_nc():
    global _NC_CACHE
    if _NC_CACHE is None:
        nc = bacc.Bacc("TRN2", target_bir_lowering=False, debug=False,
                       enable_asserts=False)
        _NC_CACHE = build(nc)
    return _NC_CACHE


def make_in_maps(s, z, z_mask, w_s, w_z, Wz, Wq, Wk, Wv, Wg, bg, Wo, bo):
    f = lambda a: np.ascontiguousarray(np.asarray(a), dtype=np.float32)
    s = f(s)
    shared = dict(s=s, w_s=f(w_s), w_z=f(w_z), Wz=f(Wz), Wq=f(Wq), Wk=f(Wk),
                  Wv=f(Wv), Wg=f(Wg), bg=f(bg), Wo=f(Wo), bo=f(bo))
    zmask = np.ascontiguousarray(np.asarray(z_mask), dtype=np.int32)
    z = f(z)
    in_maps = []
    for c in range(NCORES):
        r0, r1 = c * RB, (c + 1) * RB
        m = dict(shared)
        m["s_loc"] = np.ascontiguousarray(s[r0:r1])
        m["z"] = np.ascontiguousarray(z[r0:r1])
        m["z_mask"] = np.ascontiguousarray(zmask[r0:r1])
        in_maps.append(m)
    return in_maps


def kernel(**inputs):
    from concourse import bass_utils
    nc = _get_nc()
    in_maps = make_in_maps(**inputs)
    res = bass_utils.run_bass_kernel_spmd(nc, in_maps, core_ids=list(range(NCORES)))
    out = np.concatenate([res.results[c]["out"] for c in range(NCORES)], axis=0)
    return out.astype(np.float32)
